# revision 7
# baseline (speedup 1.0000x reference)
"""DDGCRN cell on 8 TRN2 NeuronCores — data-parallel over batch.

Per core: 8 batches. Per batch, per branch (gate O=128 / update O=64):
  hypernet MLP (transposed-feature layout) -> filt
  V = tanh(emb*time*day*speed*occupy*filt)      (10, 883)
  A = relu(V V^T)  (883,883, symmetric)  + fused row-sums (ACT accum_out)
  d = rsqrt(rowsum);  Lx = xs - d*(A @ (d*xs))  via transposed-out matmul
  out^T = bpool.T @ embT  +  sum_{e,k} wpool[e,k].T @ (embB_e * xg_k)
    (per-node weights realized as 20 accumulating K=66 matmuls; bias folded
     in as the first accumulating matmul)

All TensorE matmuls in bf16 (PSUM f32). DRAM IO f32. All PSUM tiles are
single-bank (free dim <= 512); every (., 883) product is split 512+371.
Engine ops keep all operands at partition start 0 (hardware requirement);
partition-shifted moves go through DMA.
"""

import sys, os

sys.path.insert(0, "/opt/trn_rl_repo")

import numpy as np
from contextlib import ExitStack

import concourse.bass as bass
import concourse.bacc as bacc
import concourse.mybir as mybir
from concourse import tile
from concourse.bass_utils import run_bass_kernel_spmd

AF = mybir.ActivationFunctionType
F32 = mybir.dt.float32
BF16 = mybir.dt.bfloat16

B, N, DIN, DOUT, E, CHEB = 64, 883, 2, 64, 10, 2
C = DIN + DOUT  # 66
NCORES = 8
BL = B // NCORES  # 8 batches per core
NT = (N + 127) // 128  # 7 row tiles
EKC = E * CHEB * C  # 1320 packed contraction dim
OG, OU = 2 * DOUT, DOUT  # 128, 64
SPLITS = [(0, 512), (512, N - 512)]  # psum-bank-sized free-dim splits


def _pt(nt):
    return min(128, N - nt * 128)


def _build_body(tc, ctx, nc, P):
    def pool(name, bufs, space="SBUF"):
        return ctx.enter_context(tc.tile_pool(name=name, bufs=bufs, space=space))

    wp = pool("wp", 1)        # static weights, one tag each
    stg = pool("stg", 2)      # f32 staging for casts (one shared tag)
    dat = pool("dat", 2)      # per-batch DMA loads
    act = pool("act", 2)      # per-branch intermediates
    arp = pool("arp", 8)      # relu(A) tiles
    xnp = pool("xnp", 16)     # natural xs/cand tiles (live across both branches)
    xpp = pool("xpp", 9)      # d*xs tiles
    ztp = pool("ztp", 4)      # packed z tiles (consumed immediately)
    dnp = pool("dnp", 9)      # d column pieces
    psp = pool("psp", 8, space="PSUM")  # single-bank psum tiles

    def ps_pair(name, parts, dtype=F32):
        return [
            psp.tile([parts, sl], dtype, tag="psA", name=f"{name}_{i}")
            for i, (s0, sl) in enumerate(SPLITS)
        ]

    def ps_one(name, parts, free, dtype=F32):
        return psp.tile([parts, free], dtype, tag="psA", name=name)

    def cast_in(pname, shape, tag):
        f = stg.tile([128, N], F32, tag="stg", name=pname + "_f")
        nc.sync.dma_start(f[: shape[0], : shape[1]], P[pname][:, :])
        b = wp.tile(list(shape), BF16, tag=tag, name=pname + "_b")
        nc.vector.tensor_copy(b[:, :], f[: shape[0], : shape[1]])
        return b

    # ---------------- static setup ----------------
    ident_f = wp.tile([128, 128], F32, tag="identf", name="ident_f")
    nc.sync.dma_start(ident_f[:, :], P["ident"][:, :])
    ident_b = wp.tile([128, 128], BF16, tag="identb", name="ident_b")
    nc.vector.tensor_copy(ident_b[:, :], ident_f[:, :])
    ones66 = wp.tile([1, C], BF16, tag="ones66", name="ones66")
    nc.vector.memset(ones66[:, :], 1.0)

    embT = cast_in("embT", (E, N), "embT")

    embB = []  # per e: (66, N) broadcast of emb[:, e]
    for e in range(E):
        f = stg.tile([128, N], F32, tag="stg", name=f"embB_f{e}")
        nc.sync.dma_start(f[:C, :], P["embB"][e * C : (e + 1) * C, :])
        bt = wp.tile([C, N], BF16, tag=f"embB{e}", name=f"embB{e}")
        nc.vector.tensor_copy(bt[:, :], f[:C, :])
        embB.append(bt)

    wzt = {}  # per branch: 20 chunks (66, On), chunk j = (e,k) = divmod(j,2)
    for br, On in (("g", OG), ("u", OU)):
        tiles = []
        for j in range(2 * E):
            f = stg.tile([128, N], F32, tag="stg", name=f"wz{br}_f{j}")
            nc.sync.dma_start(f[:C, :On], P[f"wz_{br}"][j * C : (j + 1) * C, :])
            bt = wp.tile([C, On], BF16, tag=f"wz{br}{j}", name=f"wz{br}{j}")
            nc.vector.tensor_copy(bt[:, :], f[:C, :On])
            tiles.append(bt)
        wzt[br] = tiles

    fc = {}
    for br in ("g", "u"):
        fc[("w1", br)] = cast_in(f"fc1w_{br}", (C, 16), f"fc1w{br}")
        fc[("w2", br)] = cast_in(f"fc2w_{br}", (16, 2), f"fc2w{br}")
        fc[("w3", br)] = cast_in(f"fc3w_{br}", (2, E), f"fc3w{br}")
        for nm, shape in (("b1", (16, 1)), ("b2", (2, 1)), ("b3", (E, 1))):
            bt = wp.tile(list(shape), F32, tag=f"fc{nm}{br}", name=f"fc{nm}{br}")
            nc.sync.dma_start(bt[:, :], P[f"fc{nm}_{br}"][:, :])
            fc[(nm, br)] = bt

    bp = {
        "g": cast_in("bpool_g", (E, OG), "bpg"),
        "u": cast_in("bpool_u", (E, OU), "bpu"),
    }

    # ---------------- per-batch ----------------
    def hypernet(br, xsT):
        """xsT (C,N) bf16 -> filtT (E,N) bf16."""
        h1p = ps_pair(f"h1p{br}", 16)
        h1 = act.tile([16, N], BF16, tag="h1", name=f"h1{br}")
        for i, (s0, sl) in enumerate(SPLITS):
            nc.tensor.matmul(h1p[i][:16, :sl], fc[("w1", br)][:, :],
                             xsT[:, s0 : s0 + sl], start=True, stop=True)
            nc.scalar.activation(h1[:, s0 : s0 + sl], h1p[i][:16, :sl],
                                 AF.Sigmoid, bias=fc[("b1", br)][:, :])
        h2p = ps_pair(f"h2p{br}", 2)
        h2 = act.tile([2, N], BF16, tag="h2", name=f"h2{br}")
        for i, (s0, sl) in enumerate(SPLITS):
            nc.tensor.matmul(h2p[i][:2, :sl], fc[("w2", br)][:, :],
                             h1[:, s0 : s0 + sl], start=True, stop=True)
            nc.scalar.activation(h2[:, s0 : s0 + sl], h2p[i][:2, :sl],
                                 AF.Sigmoid, bias=fc[("b2", br)][:, :])
        h3p = ps_pair(f"h3p{br}", E)
        filt = act.tile([E, N], BF16, tag="filt", name=f"filt{br}")
        for i, (s0, sl) in enumerate(SPLITS):
            nc.tensor.matmul(h3p[i][:E, :sl], fc[("w3", br)][:, :],
                             h2[:, s0 : s0 + sl], start=True, stop=True)
            nc.scalar.activation(filt[:, s0 : s0 + sl], h3p[i][:E, :sl],
                                 AF.Identity, bias=fc[("b3", br)][:, :])
        return filt

    def dgcn(br, b, Mb, xsT, xnat):
        """One graph-conv branch. Returns zout (On, N) bf16 = activated out^T.

        xsT (C,N) bf16 transposed feats; xnat: 7 natural (128,C) tiles.
        """
        On = OG if br == "g" else OU
        outf = AF.Sigmoid if br == "g" else AF.Tanh
        filt = hypernet(br, xsT)
        vpre = act.tile([E, N], BF16, tag="vpre", name=f"vpre{br}")
        nc.vector.tensor_mul(vpre[:, :], Mb[:, :], filt[:, :])
        V = act.tile([E, N], BF16, tag="V", name=f"V{br}")
        nc.scalar.activation(V[:, :], vpre[:, :], AF.Tanh)

        # A = relu(V V^T) tiles + fused row-sums; d = rsqrt(rowsum)
        ar, dn = [], []
        dcat = act.tile([128, 8], F32, tag="dcat", name=f"dcat{br}")
        nc.vector.memset(dcat[:, :], 0.0)
        for kt in range(NT):
            p = _pt(kt)
            aps = ps_pair(f"aps{br}{kt}", 128)
            art = arp.tile([128, N], BF16, tag="ar", name=f"ar{br}{kt}")
            rsh = []
            for i, (s0, sl) in enumerate(SPLITS):
                nc.tensor.matmul(aps[i][:p, :sl],
                                 V[:, kt * 128 : kt * 128 + p],
                                 V[:, s0 : s0 + sl], start=True, stop=True)
                rs = dnp.tile([128, 1], F32, tag=f"rs{i}", name=f"rs{br}{kt}_{i}")
                nc.scalar.activation(art[:p, s0 : s0 + sl], aps[i][:p, :sl],
                                     AF.Relu, accum_out=rs[:p, :])
                rsh.append(rs)
            rst = dnp.tile([128, 1], F32, tag="rst", name=f"rst{br}{kt}")
            nc.vector.tensor_add(rst[:p, :], rsh[0][:p, :], rsh[1][:p, :])
            dsq = dnp.tile([128, 1], F32, tag="dsq", name=f"dsq{br}{kt}")
            nc.scalar.sqrt(dsq[:p, :], rst[:p, :])
            dnt = dnp.tile([128, 1], F32, tag="dn", name=f"dn{br}{kt}")
            nc.vector.reciprocal(dnt[:p, :], dsq[:p, :])
            nc.vector.tensor_copy(dcat[:p, kt : kt + 1], dnt[:p, :])
            ar.append(art)
            dn.append(dnt)

        # x' = d * xs (natural, bf16)
        xp = []
        for kt in range(NT):
            p = _pt(kt)
            xpt = xpp.tile([128, C], BF16, tag="xp", name=f"xp{br}{kt}")
            nc.vector.tensor_scalar_mul(xpt[:p, :], xnat[kt][:p, :], dn[kt][:p, :])
            xp.append(xpt)

        # y^T = sum_kt x'[kt]^T @ Ar[kt]   (C, N) psum halves
        yt = ps_pair(f"yt{br}", C)
        for kt in range(NT):
            p = _pt(kt)
            for i, (s0, sl) in enumerate(SPLITS):
                nc.tensor.matmul(yt[i][:C, :sl], xp[kt][:p, :],
                                 ar[kt][:p, s0 : s0 + sl],
                                 start=(kt == 0), stop=(kt == NT - 1))

        # d as a broadcast row-block dB (C, N)
        tp = ps_one(f"dtp{br}", 128, 128)
        nc.tensor.transpose(tp[:8, :128], dcat[:, :], ident_f[:, :])
        drs = act.tile([8, 128], BF16, tag="drs", name=f"drs{br}")
        nc.scalar.copy(drs[:, :], tp[:8, :128])
        drow = act.tile([1, N], BF16, tag="drow", name=f"drow{br}")
        for k in range(6):
            nc.sync.dma_start(drow[0:1, k * 128 : (k + 1) * 128], drs[k : k + 1, :])
        nc.sync.dma_start(drow[0:1, 768:N], drs[6:7, 0 : N - 768])
        dB = act.tile([C, N], BF16, tag="dB", name=f"dB{br}")
        dbp = ps_pair(f"dbp{br}", C)
        for i, (s0, sl) in enumerate(SPLITS):
            nc.tensor.matmul(dbp[i][:C, :sl], ones66[:, :],
                             drow[:, s0 : s0 + sl], start=True, stop=True)
            nc.scalar.copy(dB[:, s0 : s0 + sl], dbp[i][:C, :sl])

        # Lx^T = xsT - dB * y^T
        yd = act.tile([C, N], BF16, tag="yd", name=f"yd{br}")
        for i, (s0, sl) in enumerate(SPLITS):
            nc.vector.tensor_mul(yd[:, s0 : s0 + sl], yt[i][:C, :sl],
                                 dB[:, s0 : s0 + sl])
        lxT = act.tile([C, N], BF16, tag="lxT", name=f"lxT{br}")
        nc.vector.tensor_sub(lxT[:, :], xsT[:, :], yd[:, :])

        # final acc: bias matmul first, then 20 (e,k) chunks of K=66
        xg = (xsT, lxT)
        op = ps_pair(f"op{br}", On)
        for i, (s0, sl) in enumerate(SPLITS):
            nc.tensor.matmul(op[i][:On, :sl], bp[br][:, :],
                             embT[:, s0 : s0 + sl], start=True, stop=False)
        for j in range(2 * E):
            e, k = divmod(j, 2)
            zt = ztp.tile([C, N], BF16, tag="zt", name=f"zt{br}{j}")
            nc.vector.tensor_mul(zt[:, :], embB[e][:, :], xg[k][:, :])
            for i, (s0, sl) in enumerate(SPLITS):
                nc.tensor.matmul(op[i][:On, :sl], wzt[br][j][:, :],
                                 zt[:, s0 : s0 + sl],
                                 start=False, stop=(j == 2 * E - 1))
        zout = act.tile([On, N], BF16, tag=f"zout{br}", name=f"zout{br}")
        for i, (s0, sl) in enumerate(SPLITS):
            nc.scalar.activation(zout[:, s0 : s0 + sl], op[i][:On, :sl], outf)
        return zout

    for b in range(BL):
        # natural xs tiles: [x | state] f32
        xs_nat = []
        for nt in range(NT):
            p = _pt(nt)
            t = xnp.tile([128, C], F32, tag="xsn", name=f"xsn{b}{nt}")
            nc.sync.dma_start(t[:p, 0:DIN], P["x"][b, nt * 128 : nt * 128 + p, :])
            nc.sync.dma_start(t[:p, DIN:C], P["state"][b, nt * 128 : nt * 128 + p, :])
            xs_nat.append(t)
        # transposed loads
        xsT_f = dat.tile([C, N], F32, tag="xsTf", name=f"xsTf{b}")
        nc.sync.dma_start(xsT_f[:, :], P["xsT"][b, :, :])
        xsT = act.tile([C, N], BF16, tag="xsT", name=f"xsT{b}")
        nc.vector.tensor_copy(xsT[:, :], xsT_f[:, :])
        stT_f = dat.tile([DOUT, N], F32, tag="stT", name=f"stT{b}")
        nc.sync.dma_start(stT_f[:, :], P["stateT"][b, :, :])

        # M = emb*time*day*speed*occupy (transposed, E x N)
        tdso = []
        for nm in ("tT", "dT", "sT", "oT"):
            t = dat.tile([E, N], F32, tag=nm, name=f"{nm}{b}")
            nc.sync.dma_start(t[:, :], P[nm][b, :, :])
            tdso.append(t)
        p1 = act.tile([E, N], F32, tag="p1", name=f"p1_{b}")
        nc.vector.tensor_mul(p1[:, :], tdso[0][:, :], tdso[1][:, :])
        p2 = act.tile([E, N], F32, tag="p2", name=f"p2_{b}")
        nc.vector.tensor_mul(p2[:, :], tdso[2][:, :], tdso[3][:, :])
        p3 = act.tile([E, N], F32, tag="p3", name=f"p3_{b}")
        nc.vector.tensor_mul(p3[:, :], p1[:, :], p2[:, :])
        Mb = act.tile([E, N], BF16, tag="Mb", name=f"Mb{b}")
        nc.vector.tensor_mul(Mb[:, :], p3[:, :], embT[:, :])

        # ---- gate branch ----
        zr = dgcn("g", b, Mb, xsT, xs_nat)  # (128, N): z rows 0:64, r rows 64:128

        # ---- update branch inputs ----
        # candT = [xT ; z*stateT]  assembled via DMA (partition-shifted writes)
        zst = act.tile([DOUT, N], BF16, tag="zst", name=f"zst{b}")
        nc.vector.tensor_mul(zst[:, :], zr[0:DOUT, :], stT_f[:, :])
        xTb = act.tile([DIN, N], BF16, tag="xTb", name=f"xTb{b}")
        nc.vector.tensor_copy(xTb[:, :], xsT_f[0:DIN, :])
        candT = act.tile([C, N], BF16, tag="candT", name=f"candT{b}")
        nc.sync.dma_start(candT[0:DIN, :], xTb[:, :])
        nc.sync.dma_start(candT[DIN:C, :], zst[:, :])
        # r as a start-0 tile
        r_sb = act.tile([DOUT, N], BF16, tag="r_sb", name=f"r_sb{b}")
        nc.sync.dma_start(r_sb[:, :], zr[DOUT:OG, :])

        cand_nat = []
        for nt in range(NT):
            p = _pt(nt)
            zps = ps_one(f"znp{b}{nt}", 128, 128, BF16)
            nc.tensor.transpose(zps[:p, :DOUT],
                                zr[0:DOUT, nt * 128 : nt * 128 + p],
                                ident_b[:DOUT, :DOUT])
            zn = act.tile([128, DOUT], BF16, tag="zn", name=f"zn{b}{nt}", bufs=4)
            nc.scalar.copy(zn[:p, :], zps[:p, :DOUT])
            cn = xnp.tile([128, C], BF16, tag="cn", name=f"cn{b}{nt}")
            nc.vector.tensor_copy(cn[:p, 0:DIN], xs_nat[nt][:p, 0:DIN])
            nc.vector.tensor_mul(cn[:p, DIN:C], zn[:p, :], xs_nat[nt][:p, DIN:C])
            cand_nat.append(cn)

        # ---- update branch ----
        hc = dgcn("u", b, Mb, candT, cand_nat)  # (64, N) = tanh(out_u)

        # out^T = r*state + (1-r)*hc = hc + r*(state-hc)
        t1 = act.tile([OU, N], F32, tag="t1", name=f"t1_{b}", bufs=1)
        nc.vector.tensor_sub(t1[:, :], stT_f[:, :], hc[:, :])
        t2 = act.tile([OU, N], F32, tag="t2", name=f"t2_{b}", bufs=1)
        nc.vector.tensor_mul(t2[:, :], r_sb[:, :], t1[:, :])
        outT = act.tile([OU, N], F32, tag="outT", name=f"outT{b}")
        nc.vector.tensor_add(outT[:, :], t2[:, :], hc[:, :])

        # transpose back to natural and DMA out
        for nt in range(NT):
            p = _pt(nt)
            tp = ps_one(f"otp{b}{nt}", 128, 128)
            nc.tensor.transpose(tp[:p, :DOUT],
                                outT[:, nt * 128 : nt * 128 + p],
                                ident_f[:DOUT, :DOUT])
            onat = act.tile([128, DOUT], F32, tag="onat", name=f"onat{b}{nt}", bufs=4)
            nc.scalar.copy(onat[:p, :], tp[:p, :DOUT])
            nc.sync.dma_start(P["out"][b, nt * 128 : nt * 128 + p, :], onat[:p, :])


def build_nc():
    nc = bacc.Bacc()
    P = {}

    def dp(name, shape, out=False):
        P[name] = nc.declare_dram_parameter(name, list(shape), F32, isOutput=out)

    dp("x", (BL, N, DIN))
    dp("state", (BL, N, DOUT))
    dp("xsT", (BL, C, N))
    dp("stateT", (BL, DOUT, N))
    for nm in ("tT", "dT", "sT", "oT"):
        dp(nm, (BL, E, N))
    dp("embT", (E, N))
    dp("embB", (E * C, N))
    dp("wz_g", (EKC, OG))
    dp("wz_u", (EKC, OU))
    dp("bpool_g", (E, OG))
    dp("bpool_u", (E, OU))
    for br in ("g", "u"):
        dp(f"fc1w_{br}", (C, 16))
        dp(f"fc2w_{br}", (16, 2))
        dp(f"fc3w_{br}", (2, E))
        dp(f"fcb1_{br}", (16, 1))
        dp(f"fcb2_{br}", (2, 1))
        dp(f"fcb3_{br}", (E, 1))
    dp("ident", (128, 128))
    dp("out", (BL, N, DOUT), out=True)
    with tile.TileContext(nc) as tc:
        with ExitStack() as ctx:
            _build_body(tc, ctx, nc, P)
    nc.finalize()
    return nc


_NC_CACHE = {}


def _get_nc():
    if "nc" not in _NC_CACHE:
        _NC_CACHE["nc"] = build_nc()
    return _NC_CACHE["nc"]


def _make_in_maps(inputs):
    f = lambda a: np.ascontiguousarray(a, dtype=np.float32)
    x = f(inputs["x"])
    state = f(inputs["state"])
    emb = f(inputs["node_embeddings"])
    time, day = f(inputs["time"]), f(inputs["day"])
    speed, occupy = f(inputs["speed"]), f(inputs["occupy"])
    xs = np.concatenate([x, state], axis=-1)  # (B, N, C)

    shared = {
        "embT": f(emb.T),
        "embB": f(np.repeat(emb.T[:, None, :], C, axis=1).reshape(E * C, N)),
        "wz_g": f(inputs["gate_wpool"].reshape(EKC, OG)),
        "wz_u": f(inputs["update_wpool"].reshape(EKC, OU)),
        "bpool_g": f(inputs["gate_bpool"]),
        "bpool_u": f(inputs["update_bpool"]),
        "ident": np.eye(128, dtype=np.float32),
    }
    for br, pre in (("g", "gate"), ("u", "update")):
        shared[f"fc1w_{br}"] = f(inputs[f"{pre}_fc1_w"])
        shared[f"fc2w_{br}"] = f(inputs[f"{pre}_fc2_w"])
        shared[f"fc3w_{br}"] = f(inputs[f"{pre}_fc3_w"])
        shared[f"fcb1_{br}"] = f(inputs[f"{pre}_fc1_b"].reshape(16, 1))
        shared[f"fcb2_{br}"] = f(inputs[f"{pre}_fc2_b"].reshape(2, 1))
        shared[f"fcb3_{br}"] = f(inputs[f"{pre}_fc3_b"].reshape(E, 1))

    in_maps = []
    for c in range(NCORES):
        sl = slice(c * BL, (c + 1) * BL)
        m = dict(shared)
        m["x"] = x[sl]
        m["state"] = state[sl]
        m["xsT"] = f(xs[sl].transpose(0, 2, 1))
        m["stateT"] = f(state[sl].transpose(0, 2, 1))
        m["tT"] = f(time[sl].transpose(0, 2, 1))
        m["dT"] = f(day[sl].transpose(0, 2, 1))
        m["sT"] = f(speed[sl].transpose(0, 2, 1))
        m["oT"] = f(occupy[sl].transpose(0, 2, 1))
        in_maps.append(m)
    return in_maps


def _run(inputs, trace=False):
    nc = _get_nc()
    in_maps = _make_in_maps(inputs)
    res = run_bass_kernel_spmd(nc, in_maps, core_ids=list(range(NCORES)), trace=trace)
    out = np.concatenate([np.asarray(res.results[i]["out"]) for i in range(NCORES)], axis=0)
    return out.astype(np.float32), res


def kernel(**inputs):
    out, _ = _run(inputs, trace=False)
    return out
